# revision 44
# baseline (speedup 1.0000x reference)
"""Cosine cross-attention (B=4, L=2048, D=1024, H=16, dh=64, tau=0.07) on 8 trn2 cores.

Sharding: core = b*2 + g  (b in 0..3 data-parallel, g in 0..1 head-group of 8 heads).
Per core everything is computed feature-major ("T" = transposed, [feature, L]):
  QT = l2norm-by-head( wq.T @ xqT + bq )        [512, 2048]  (normalization via
      PE-broadcast of 1/||q|| and in-place DVE multiply)
  KT =                 wk.T @ xkT + bk          [512, 2048]  (its 1/||k||/tau goes
      into the per-partition scale of the exp activation)
  V  natural layout    (xvT.T chunks) @ wv      [2048, 512]  no bias (bv folded
      into a host-side output bias: softmax rows sum to 1)
  per head pair (m) / head (s):   S.T tile = KT_h.T-chunk.T @ QT-block  (K=64,
      auto row-tiled 64x128 so the two heads share the PE array; K itself is
      pre-scaled by rnk/tau so the exp needs no per-partition scale and one
      ACT call covers both heads)
  E.T = exp(S.T)  on ACT, psum->sbuf f32r
  OT  = [V | 1].T @ E.T  accumulated over Lk in PSUM -> row 64 is the softmax Z
  MT  = OT[0:64]/Z  via DVE reciprocal + PE-broadcast + in-place multiply
  OUT.T partial = wo-chunks.T @ MT-chunks       [1024, 2048]
Host: out[b] = (partial_g0 + partial_g1).T + (bo + bv @ Wo.T).

All matmuls run as float32r (tf32-like, 1 cycle/row at N>=256).
"""

import numpy as np

import concourse.bacc as bacc
import concourse.tile as tile
from concourse import mybir
from concourse.bass_utils import run_bass_kernel_spmd

P = 128
L = 2048
D = 1024
DO = 512  # per-core output dims of q/k/v projections (8 heads * 64)
TAU = 0.07
NLB = L // 512   # 4 blocks of 512 along L
NLK = L // 128   # 16 chunks of 128 along L (keys)
NM = DO // P     # 4 dout chunks (head pairs)
NKC = D // P     # 8 contraction chunks for projections

F32 = mybir.dt.float32
F32R = mybir.dt.float32r
BF16 = mybir.dt.bfloat16
EXP = mybir.ActivationFunctionType.Exp
SQRT = mybir.ActivationFunctionType.Sqrt
SQUARE = mybir.ActivationFunctionType.Square
MULT = mybir.AluOpType.mult

_CACHE = {}
VARIANT = None


def _emit(nc, prm, repeat=1, phases="abcd"):
    from contextlib import ExitStack
    with tile.TileContext(nc) as tc:
        if repeat > 1:
            with tc.For_i(0, repeat, 1):
                _emit_body(nc, tc, prm, phases)
        else:
            _emit_body(nc, tc, prm, phases)


def _emit_body(nc, tc, prm, phases="abcd"):
    from contextlib import ExitStack
    with ExitStack() as stack:
        const = stack.enter_context(tc.tile_pool(name="const", bufs=1))
        persist = stack.enter_context(tc.tile_pool(name="persist", bufs=1))
        normp = stack.enter_context(tc.tile_pool(name="normp", bufs=2))

        indt = const.tile([P, 2], F32R, tag="indt")
        nc.sync.dma_start(out=indt[:], in_=prm["indt"][:])
        ones8 = const.tile([P, 8], F32R, tag="ones8")
        nc.sync.dma_start(out=ones8[:], in_=prm["ones8"][:])
        selq = const.tile([8, NM, P], F32R, tag="selq")
        nc.sync.dma_start(out=selq[:], in_=prm["selq"][:])
        selz = const.tile([8, 8, 64], F32R, tag="selz")
        nc.sync.dma_start(out=selz[:], in_=prm["selz"][:])
        bq_t = const.tile([P, NM], F32, tag="bq")
        bk_t = const.tile([P, NM], F32, tag="bk")
        for m in range(NM):
            nc.sync.dma_start(out=bq_t[:, m], in_=prm["bq"][m * P:(m + 1) * P])
            nc.sync.dma_start(out=bk_t[:, m], in_=prm["bk"][m * P:(m + 1) * P])

        qt = [persist.tile([P, L], F32R, tag=f"qt{m}", name=f"qt{m}") for m in range(NM)]
        kt = [persist.tile([P, L], F32R, tag=f"kt{m}", name=f"kt{m}") for m in range(NM)]
        vg_all = persist.tile([P, NLK, 8, 65], F32R, tag="vg_all")
        vg = [vg_all[:, i] for i in range(NLK)]
        nq_all = persist.tile([8, L], F32R, tag="nq_all")
        nk_all = persist.tile([8, L], F32R, tag="nk_all")

        # ---------------- Phase A: projections ----------------
        with tc.tile_pool(name="wp", bufs=1) as wp, \
             tc.tile_pool(name="xp", bufs=3) as xp, \
             tc.tile_pool(name="sqp", bufs=1) as sqp, \
             tc.tile_pool(name="psA", bufs=1, space="PSUM") as psA, \
             tc.tile_pool(name="psN", bufs=2, space="PSUM") as psN, \
             tc.tile_pool(name="psBC", bufs=1, space="PSUM") as psBC:

            for kind in (("q", "k", "v") if "a" in phases else ()):
                w_d = prm["w" + kind]
                x_d = prm["x" + kind]
                wt = []
                for kc in range(NKC):
                    w_t = wp.tile([P, DO], F32R, tag=f"w{kc}")
                    nc.sync.dma_start(out=w_t[:], in_=w_d[kc * P:(kc + 1) * P, :])
                    wt.append(w_t)
                for lb in range(NLB):
                    pas = [psA.tile([P, 512], F32, tag=f"pa{j}", name=f"pa{j}") for j in range(NM)]
                    for kc2 in range(NKC // 2):
                        # paired-chunk load: one DMA brings two contraction chunks
                        x_t = xp.tile([P, 2, 512], F32R, tag="x")
                        nc.sync.dma_start(
                            out=x_t[:],
                            in_=x_d[2 * kc2 * P:(2 * kc2 + 2) * P,
                                    lb * 512:(lb + 1) * 512].rearrange(
                                        "(two p) i -> p two i", two=2))
                        for half in range(2):
                            kc = 2 * kc2 + half
                            xv = x_t[:, half, :]
                            if kind == "v":
                                for j in range(NM):
                                    nc.tensor.matmul(
                                        pas[j][:], lhsT=xv[:, j * P:(j + 1) * P], rhs=wt[kc][:],
                                        start=(kc == 0), stop=(kc == NKC - 1))
                            else:
                                for m in range(NM):
                                    nc.tensor.matmul(
                                        pas[m][:], lhsT=wt[kc][:, m * P:(m + 1) * P], rhs=xv,
                                        start=(kc == 0), stop=(kc == NKC - 1))
                    if kind == "v":
                        for j in range(NM):
                            lc = lb * 4 + j
                            nc.vector.tensor_copy(
                                out=vg[lc][:, :, 0:64],
                                in_=pas[j][:].rearrange("p (h d) -> p h d", h=8))
                            nc.vector.tensor_copy(out=vg[lc][:, :, 64], in_=ones8[:])
                    else:
                        b_t = bq_t if kind == "q" else bk_t
                        n_all = nq_all if kind == "q" else nk_all
                        for m in range(NM):
                            sl = slice(lb * 512, (lb + 1) * 512)
                            blk = (qt if kind == "q" else kt)[m][:, sl]
                            nc.vector.tensor_scalar_add(
                                out=blk, in0=pas[m][:], scalar1=b_t[:, m:m + 1])
                            sq_t = sqp.tile([P, 512], F32R, tag="sq")
                            # (x+b)^2 in one ACT op -- ScalarE is idle in phase A
                            nc.scalar.activation(out=sq_t[:], in_=pas[m][:], func=SQUARE,
                                                 bias=b_t[:, m:m + 1])
                            nqp = psN.tile([2, 512], F32, tag="nq")
                            nc.tensor.matmul(nqp[:], lhsT=indt[:], rhs=sq_t[:],
                                             start=True, stop=True)
                            nqb = normp.tile([2, 512], F32R, tag="nqb", bufs=1)
                            nc.vector.tensor_copy(out=nqb[:], in_=nqp[:])
                            nc.sync.dma_start(
                                out=n_all[2 * m:2 * m + 2, lb * 512:(lb + 1) * 512],
                                in_=nqb[:])

            # ---------------- Phase B: norms ----------------
            if "b" in phases:
                _emit_norms(nc, tc, normp, psBC, qt, kt, nq_all, nk_all, selq)

        # prefetch out-projection weights so phase D starts immediately
        wop = stack.enter_context(tc.tile_pool(name="wop", bufs=1))
        wot = []
        for kc in range(NM):
            w_t = wop.tile([P, D], F32R, tag=f"wo{kc}", name=f"wo{kc}")
            nc.sync.dma_start(out=w_t[:], in_=prm["wo"][kc * P:(kc + 1) * P, :])
            wot.append(w_t)

        # ---------------- Phase C: attention ----------------
        mtp = stack.enter_context(tc.tile_pool(name="mtp", bufs=1))
        mt = [mtp.tile([P, L], F32R, tag=f"mt{m}", name=f"mt{m}") for m in range(NM)]
        with tc.tile_pool(name="psS", bufs=2, space="PSUM") as psS, \
             tc.tile_pool(name="psOT", bufs=2, space="PSUM") as psOT, \
             tc.tile_pool(name="etp", bufs=4) as etp:
            for m in range(NM if "c" in phases else 0):
                zpack = normp.tile([8, 512], F32R, tag="zp", bufs=1)
                for lq in range(NLB):
                    ot0 = psOT.tile([65, 512], F32, tag="ot0")
                    ot1 = psOT.tile([65, 512], F32, tag="ot1")
                    for lk in range(NLK):
                        pss = psS.tile([P, 1024], F32, tag="pss")
                        # the two heads run in opposite PE array halves
                        for s in range(2):
                            base = s * 64
                            nc.tensor.matmul(
                                pss[:, s * 512:(s + 1) * 512],
                                lhsT=kt[m][base:base + 64, lk * P:(lk + 1) * P],
                                rhs=qt[m][base:base + 64, lq * 512:(lq + 1) * 512],
                                start=True, stop=True)
                        et = etp.tile([P, 1024], F32R, tag="et")
                        nc.scalar.activation(out=et[:], in_=pss[:], func=EXP)
                        _emit_pv(nc, vg, et, lk, ot0, ot1, m)
                    for s, ot in enumerate((ot0, ot1)):
                        nc.vector.tensor_copy(
                            out=mt[m][s * 64:s * 64 + 64, lq * 512:(lq + 1) * 512],
                            in_=ot[0:64, :])
                        zb = normp.tile([1, 512], F32R, tag="zb", bufs=1)
                        nc.vector.tensor_copy(out=zb[:], in_=ot[64:65, :])
                        r = s * 4 + lq
                        nc.sync.dma_start(out=zpack[r:r + 1, :], in_=zb[:])
                rz = zpack
                with nc.allow_low_precision(reason="f32r reciprocal, tf32 rounding is fine here"):
                    nc.vector.reciprocal(out=rz[:], in_=zpack[:])
                for s in range(2):
                    for lq in range(NLB):
                        r = s * 4 + lq
                        # borrow a pss slot for the Z broadcast (the score
                        # pipeline drains at the m boundary anyway)
                        bc = psS.tile([P, 1024], F32, tag="pss")
                        nc.tensor.matmul(bc[0:64, 0:512], lhsT=selz[:, r, :], rhs=rz[:],
                                         start=True, stop=True)
                        blk = mt[m][s * 64:s * 64 + 64, lq * 512:(lq + 1) * 512]
                        nc.vector.tensor_tensor(out=blk, in0=blk, in1=bc[0:64, 0:512],
                                                op=MULT)

        # ---------------- Phase D: output projection ----------------
        if "d" not in phases:
            ob0 = normp.tile([P, 512], F32, tag="dummyout")
            nc.vector.memset(ob0[:], 0.0)
            nc.sync.dma_start(out=prm["out_t"][0:P, 0:512], in_=ob0[:])
            return
        with tc.tile_pool(name="obp", bufs=4) as obp, \
             tc.tile_pool(name="psD", bufs=4, space="PSUM") as psD:
            for mo in range(D // P):
                for lb in range(NLB):
                    pd = psD.tile([P, 512], F32, tag="pd")
                    for kc in range(NM):
                        nc.tensor.matmul(pd[:], lhsT=wot[kc][:, mo * P:(mo + 1) * P],
                                         rhs=mt[kc][:, lb * 512:(lb + 1) * 512],
                                         start=(kc == 0), stop=(kc == NM - 1))
                    ob = obp.tile([P, 512], F32, tag="ob")
                    nc.vector.tensor_copy(out=ob[:], in_=pd[:])
                    nc.sync.dma_start(
                        out=prm["out_t"][mo * P:(mo + 1) * P, lb * 512:(lb + 1) * 512],
                        in_=ob[:])



def build_nc(repeat=1, phases="abcd"):
    key = (repeat, phases)
    if key in _CACHE:
        return _CACHE[key]
    nc = bacc.Bacc("TRN2", target_bir_lowering=False, debug=False, num_devices=8)
    prm = {}
    for name in ("xq", "xk", "xv"):
        prm[name] = nc.declare_dram_parameter(name, [D, L], F32R, isOutput=False)
    for name in ("wq", "wk", "wv"):
        prm[name] = nc.declare_dram_parameter(name, [D, DO], F32R, isOutput=False)
    prm["wo"] = nc.declare_dram_parameter("wo", [DO, D], F32R, isOutput=False)
    prm["bq"] = nc.declare_dram_parameter("bq", [DO], F32, isOutput=False)
    prm["bk"] = nc.declare_dram_parameter("bk", [DO], F32, isOutput=False)
    prm["indt"] = nc.declare_dram_parameter("indt", [P, 2], F32R, isOutput=False)
    prm["ones8"] = nc.declare_dram_parameter("ones8", [P, 8], F32R, isOutput=False)
    prm["selq"] = nc.declare_dram_parameter("selq", [8, NM, P], F32R, isOutput=False)
    prm["selz"] = nc.declare_dram_parameter("selz", [8, 8, 64], F32R, isOutput=False)
    prm["out_t"] = nc.declare_dram_parameter("out_t", [D, L], F32, isOutput=True)
    _emit(nc, prm, repeat=repeat, phases=phases)
    nc.compile()
    _CACHE[key] = nc
    return nc


def make_in_maps(q, k, v, Wq, bq, Wk, bk, Wv, bv, Wo, bo):
    B = q.shape[0]
    f32 = np.float32

    indt = np.zeros((P, 2), f32)
    indt[0:64, 0] = 1.0
    indt[64:128, 1] = 1.0
    ones8 = np.ones((P, 8), f32)
    selq = np.zeros((8, NM, P), f32)
    for m in range(NM):
        for j in range(P):
            selq[2 * m + j // 64, m, j] = 1.0
    selz = np.zeros((8, 8, 64), f32)
    for r in range(8):
        selz[r, r, :] = 1.0

    in_maps = []
    for b in range(B):
        for g in range(2):
            sl = slice(g * DO, (g + 1) * DO)
            in_maps.append({
                "xq": np.ascontiguousarray(q[b].T.astype(f32)),
                "xk": np.ascontiguousarray(k[b].T.astype(f32)),
                "xv": np.ascontiguousarray(v[b].T.astype(f32)),
                "wq": np.ascontiguousarray(Wq[sl, :].T.astype(f32)),
                "wk": np.ascontiguousarray(Wk[sl, :].T.astype(f32)),
                "wv": np.ascontiguousarray(Wv[sl, :].T.astype(f32)),
                "wo": np.ascontiguousarray(Wo[:, sl].T.astype(f32)),
                "bq": np.ascontiguousarray(bq[sl].astype(f32)),
                "bk": np.ascontiguousarray(bk[sl].astype(f32)),
                "indt": indt, "ones8": ones8, "selq": selq, "selz": selz,
            })
    return in_maps


def assemble(results, bv, Wo, bo):
    B = len(results) // 2
    bias = (bo + bv @ Wo.T).astype(np.float32)
    outs = []
    for b in range(B):
        part = results[2 * b]["out_t"] + results[2 * b + 1]["out_t"]
        outs.append(part.T + bias)
    return np.stack(outs).astype(np.float32)


def kernel(q, k, v, Wq, bq, Wk, bk, Wv, bv, Wo, bo):
    q, k, v = (np.asarray(t, np.float32) for t in (q, k, v))
    Wq, bq, Wk, bk, Wv, bv, Wo, bo = (
        np.asarray(t, np.float32) for t in (Wq, bq, Wk, bk, Wv, bv, Wo, bo))
    nc = build_nc()
    in_maps = make_in_maps(q, k, v, Wq, bq, Wk, bk, Wv, bv, Wo, bo)
    last_err = None
    for attempt in range(3):
        try:
            res = run_bass_kernel_spmd(nc, in_maps, core_ids=list(range(8)))
            return assemble(res.results, bv, Wo, bo)
        except Exception as e:  # transient NRT device errors: retry
            last_err = e
            import time as _time
            _time.sleep(2.0)
    raise last_err


def _emit_norms(nc, tc, normp, psBC, qt, kt, nq_all, nk_all, selq):
    with nc.allow_low_precision(reason="f32r norm chain, tf32 rounding fine"):
        nc.scalar.activation(out=nq_all[:], in_=nq_all[:], func=SQRT)
        nc.vector.tensor_scalar_max(out=nq_all[:], in0=nq_all[:], scalar1=1e-12)
        nc.vector.reciprocal(out=nq_all[:], in_=nq_all[:])
        nc.scalar.activation(out=nk_all[:], in_=nk_all[:], func=SQRT)
        # clamp at eps, then fold the softmax temperature into k's norm
        nc.vector.tensor_scalar_max(out=nk_all[:], in0=nk_all[:], scalar1=1e-12)
        nc.vector.tensor_scalar_mul(out=nk_all[:], in0=nk_all[:], scalar1=TAU)
        nc.vector.reciprocal(out=nk_all[:], in_=nk_all[:])

    # normalize Q and K in place via PE-broadcast of the row pair
    for which, r_all in (("q", nq_all), ("k", nk_all)):
        for m in range(NM):
            for lb in range(NLB):
                sl = slice(lb * 512, (lb + 1) * 512)
                bc = psBC.tile([P, 512], F32, tag="bcq")
                nc.tensor.matmul(bc[:], lhsT=selq[:, m, :], rhs=r_all[:, sl],
                                 start=True, stop=True)
                blk = (qt if which == "q" else kt)[m][:, sl]
                nc.vector.tensor_tensor(out=blk, in0=blk, in1=bc[:], op=MULT)


def _emit_pv(nc, vg, et, lk, ot0, ot1, m):
    nc.tensor.matmul(ot0[:], lhsT=vg[lk][:, 2 * m, :], rhs=et[:, 0:512],
                     start=(lk == 0), stop=(lk == NLK - 1), skip_group_check=True)
    nc.tensor.matmul(ot1[:], lhsT=vg[lk][:, 2 * m + 1, :], rhs=et[:, 512:1024],
                     start=(lk == 0), stop=(lk == NLK - 1), skip_group_check=True)



# revision 45
# speedup vs baseline: 1.1277x; 1.1277x over previous
"""Cosine cross-attention (B=4, L=2048, D=1024, H=16, dh=64, tau=0.07) on 8 trn2 cores.

Sharding: core = b*2 + g  (b in 0..3 data-parallel, g in 0..1 head-group of 8 heads).
Per core everything is computed feature-major ("T" = transposed, [feature, L]):
  QT = l2norm-by-head( wq.T @ xqT + bq )        [512, 2048]  (normalization via
      PE-broadcast of 1/||q|| and in-place DVE multiply)
  KT =                 wk.T @ xkT + bk          [512, 2048]  (its 1/||k||/tau goes
      into the per-partition scale of the exp activation)
  V  natural layout    (xvT.T chunks) @ wv      [2048, 512]  no bias (bv folded
      into a host-side output bias: softmax rows sum to 1)
  per head pair (m) / head (s):   S.T tile = KT_h.T-chunk.T @ QT-block  (K=64,
      auto row-tiled 64x128 so the two heads share the PE array; K itself is
      pre-scaled by rnk/tau so the exp needs no per-partition scale and one
      ACT call covers both heads)
  E.T = exp(S.T)  on ACT, psum->sbuf f32r
  OT  = [V | 1].T @ E.T  accumulated over Lk in PSUM -> row 64 is the softmax Z
  MT  = OT[0:64]/Z  via DVE reciprocal + PE-broadcast + in-place multiply
  OUT.T partial = wo-chunks.T @ MT-chunks       [1024, 2048]
Host: out[b] = (partial_g0 + partial_g1).T + (bo + bv @ Wo.T).

All matmuls run as float32r (tf32-like, 1 cycle/row at N>=256).
"""

import numpy as np

import concourse.bacc as bacc
import concourse.tile as tile
from concourse import mybir
from concourse.bass_utils import run_bass_kernel_spmd

P = 128
L = 2048
D = 1024
DO = 512  # per-core output dims of q/k/v projections (8 heads * 64)
TAU = 0.07
NLB = L // 512   # 4 blocks of 512 along L
NLK = L // 128   # 16 chunks of 128 along L (keys)
NM = DO // P     # 4 dout chunks (head pairs)
NKC = D // P     # 8 contraction chunks for projections

F32 = mybir.dt.float32
F32R = mybir.dt.float32r
BF16 = mybir.dt.bfloat16
EXP = mybir.ActivationFunctionType.Exp
SQRT = mybir.ActivationFunctionType.Sqrt
SQUARE = mybir.ActivationFunctionType.Square
MULT = mybir.AluOpType.mult

_CACHE = {}
VARIANT = None


def _emit(nc, prm, repeat=1, phases="abcd"):
    from contextlib import ExitStack
    with tile.TileContext(nc) as tc:
        if repeat > 1:
            with tc.For_i(0, repeat, 1):
                _emit_body(nc, tc, prm, phases)
        else:
            _emit_body(nc, tc, prm, phases)


def _emit_body(nc, tc, prm, phases="abcd"):
    from contextlib import ExitStack
    with ExitStack() as stack:
        const = stack.enter_context(tc.tile_pool(name="const", bufs=1))
        persist = stack.enter_context(tc.tile_pool(name="persist", bufs=1))
        normp = stack.enter_context(tc.tile_pool(name="normp", bufs=2))

        indt = const.tile([P, 2], F32R, tag="indt")
        nc.sync.dma_start(out=indt[:], in_=prm["indt"][:])
        ones8 = const.tile([P, 8], F32R, tag="ones8")
        nc.sync.dma_start(out=ones8[:], in_=prm["ones8"][:])
        selq = const.tile([8, NM, P], F32R, tag="selq")
        nc.sync.dma_start(out=selq[:], in_=prm["selq"][:])
        selz = const.tile([8, 8, 64], F32R, tag="selz")
        nc.sync.dma_start(out=selz[:], in_=prm["selz"][:])
        bq_t = const.tile([P, NM], F32, tag="bq")
        bk_t = const.tile([P, NM], F32, tag="bk")
        for m in range(NM):
            nc.sync.dma_start(out=bq_t[:, m], in_=prm["bq"][m * P:(m + 1) * P])
            nc.sync.dma_start(out=bk_t[:, m], in_=prm["bk"][m * P:(m + 1) * P])

        qt = [persist.tile([P, L], F32R, tag=f"qt{m}", name=f"qt{m}") for m in range(NM)]
        kt = [persist.tile([P, L], F32R, tag=f"kt{m}", name=f"kt{m}") for m in range(NM)]
        vg_all = persist.tile([P, NLK, 8, 65], F32R, tag="vg_all")
        vg = [vg_all[:, i] for i in range(NLK)]
        nq_all = persist.tile([8, L], F32R, tag="nq_all")
        nk_all = persist.tile([8, L], F32R, tag="nk_all")

        # ---------------- Phase A: projections ----------------
        with tc.tile_pool(name="wp", bufs=1) as wp, \
             tc.tile_pool(name="xp", bufs=3) as xp, \
             tc.tile_pool(name="sqp", bufs=1) as sqp, \
             tc.tile_pool(name="psA", bufs=1, space="PSUM") as psA, \
             tc.tile_pool(name="psN", bufs=1, space="PSUM") as psN, \
             tc.tile_pool(name="psBC", bufs=2, space="PSUM") as psBC:

            for kind in (("q", "k", "v") if "a" in phases else ()):
                w_d = prm["w" + kind]
                x_d = prm["x" + kind]
                wt = []
                for kc in range(NKC):
                    w_t = wp.tile([P, DO], F32R, tag=f"w{kc}")
                    nc.sync.dma_start(out=w_t[:], in_=w_d[kc * P:(kc + 1) * P, :])
                    wt.append(w_t)
                for lb in range(NLB):
                    pas = [psA.tile([P, 512], F32, tag=f"pa{j}", name=f"pa{j}") for j in range(NM)]
                    for kc2 in range(NKC // 2):
                        # paired-chunk load: one DMA brings two contraction chunks
                        x_t = xp.tile([P, 2, 512], F32R, tag="x")
                        nc.sync.dma_start(
                            out=x_t[:],
                            in_=x_d[2 * kc2 * P:(2 * kc2 + 2) * P,
                                    lb * 512:(lb + 1) * 512].rearrange(
                                        "(two p) i -> p two i", two=2))
                        for half in range(2):
                            kc = 2 * kc2 + half
                            xv = x_t[:, half, :]
                            if kind == "v":
                                for j in range(NM):
                                    nc.tensor.matmul(
                                        pas[j][:], lhsT=xv[:, j * P:(j + 1) * P], rhs=wt[kc][:],
                                        start=(kc == 0), stop=(kc == NKC - 1))
                            else:
                                for m in range(NM):
                                    nc.tensor.matmul(
                                        pas[m][:], lhsT=wt[kc][:, m * P:(m + 1) * P], rhs=xv,
                                        start=(kc == 0), stop=(kc == NKC - 1))
                    if kind == "v":
                        for j in range(NM):
                            lc = lb * 4 + j
                            nc.vector.tensor_copy(
                                out=vg[lc][:, :, 0:64],
                                in_=pas[j][:].rearrange("p (h d) -> p h d", h=8))
                            nc.vector.tensor_copy(out=vg[lc][:, :, 64], in_=ones8[:])
                    else:
                        b_t = bq_t if kind == "q" else bk_t
                        n_all = nq_all if kind == "q" else nk_all
                        for m in range(NM):
                            sl = slice(lb * 512, (lb + 1) * 512)
                            blk = (qt if kind == "q" else kt)[m][:, sl]
                            nc.vector.tensor_scalar_add(
                                out=blk, in0=pas[m][:], scalar1=b_t[:, m:m + 1])
                            sq_t = sqp.tile([P, 512], F32R, tag="sq")
                            # (x+b)^2 in one ACT op -- ScalarE is idle in phase A
                            nc.scalar.activation(out=sq_t[:], in_=pas[m][:], func=SQUARE,
                                                 bias=b_t[:, m:m + 1])
                            nqp = psN.tile([2, 512], F32, tag="nq")
                            nc.tensor.matmul(nqp[:], lhsT=indt[:], rhs=sq_t[:],
                                             start=True, stop=True)
                            nqb = normp.tile([2, 512], F32R, tag="nqb", bufs=1)
                            nc.vector.tensor_copy(out=nqb[:], in_=nqp[:])
                            nc.sync.dma_start(
                                out=n_all[2 * m:2 * m + 2, lb * 512:(lb + 1) * 512],
                                in_=nqb[:])

            # ---------------- Phase B: norms ----------------
            if "b" in phases:
                _emit_norms(nc, tc, normp, psBC, qt, kt, nq_all, nk_all, selq)

        # prefetch out-projection weights so phase D starts immediately
        wop = stack.enter_context(tc.tile_pool(name="wop", bufs=1))
        wot = []
        for kc in range(NM):
            w_t = wop.tile([P, D], F32R, tag=f"wo{kc}", name=f"wo{kc}")
            nc.sync.dma_start(out=w_t[:], in_=prm["wo"][kc * P:(kc + 1) * P, :])
            wot.append(w_t)

        # ---------------- Phase C: attention ----------------
        mtp = stack.enter_context(tc.tile_pool(name="mtp", bufs=1))
        mt = [mtp.tile([P, L], F32R, tag=f"mt{m}", name=f"mt{m}") for m in range(NM)]
        with tc.tile_pool(name="psS", bufs=2, space="PSUM") as psS, \
             tc.tile_pool(name="psOT", bufs=2, space="PSUM") as psOT, \
             tc.tile_pool(name="etp", bufs=4) as etp:
            for m in range(NM if "c" in phases else 0):
                zpack = normp.tile([8, 512], F32R, tag="zp", bufs=1)
                for lq in range(NLB):
                    ot0 = psOT.tile([65, 512], F32, tag="ot0")
                    ot1 = psOT.tile([65, 512], F32, tag="ot1")
                    for lk in range(NLK):
                        pss = psS.tile([P, 1024], F32, tag="pss")
                        # the two heads run in opposite PE array halves
                        for s in range(2):
                            base = s * 64
                            nc.tensor.matmul(
                                pss[:, s * 512:(s + 1) * 512],
                                lhsT=kt[m][base:base + 64, lk * P:(lk + 1) * P],
                                rhs=qt[m][base:base + 64, lq * 512:(lq + 1) * 512],
                                start=True, stop=True)
                        et = etp.tile([P, 1024], F32R, tag="et")
                        nc.scalar.activation(out=et[:], in_=pss[:], func=EXP)
                        _emit_pv(nc, vg, et, lk, ot0, ot1, m)
                    for s, ot in enumerate((ot0, ot1)):
                        nc.vector.tensor_copy(
                            out=mt[m][s * 64:s * 64 + 64, lq * 512:(lq + 1) * 512],
                            in_=ot[0:64, :])
                        zb = normp.tile([1, 512], F32R, tag="zb", bufs=1)
                        nc.vector.tensor_copy(out=zb[:], in_=ot[64:65, :])
                        r = s * 4 + lq
                        nc.sync.dma_start(out=zpack[r:r + 1, :], in_=zb[:])
                rz = zpack
                with nc.allow_low_precision(reason="f32r reciprocal, tf32 rounding is fine here"):
                    nc.vector.reciprocal(out=rz[:], in_=zpack[:])
                for s in range(2):
                    for lq in range(NLB):
                        r = s * 4 + lq
                        # borrow a pss slot for the Z broadcast (the score
                        # pipeline drains at the m boundary anyway)
                        bc = psS.tile([P, 1024], F32, tag="pss")
                        nc.tensor.matmul(bc[0:64, 0:512], lhsT=selz[:, r, :], rhs=rz[:],
                                         start=True, stop=True)
                        blk = mt[m][s * 64:s * 64 + 64, lq * 512:(lq + 1) * 512]
                        nc.vector.tensor_tensor(out=blk, in0=blk, in1=bc[0:64, 0:512],
                                                op=MULT)

        # ---------------- Phase D: output projection ----------------
        if "d" not in phases:
            ob0 = normp.tile([P, 512], F32, tag="dummyout")
            nc.vector.memset(ob0[:], 0.0)
            nc.sync.dma_start(out=prm["out_t"][0:P, 0:512], in_=ob0[:])
            return
        with tc.tile_pool(name="obp", bufs=4) as obp, \
             tc.tile_pool(name="psD", bufs=4, space="PSUM") as psD:
            for mo in range(D // P):
                for lb in range(NLB):
                    pd = psD.tile([P, 512], F32, tag="pd")
                    for kc in range(NM):
                        nc.tensor.matmul(pd[:], lhsT=wot[kc][:, mo * P:(mo + 1) * P],
                                         rhs=mt[kc][:, lb * 512:(lb + 1) * 512],
                                         start=(kc == 0), stop=(kc == NM - 1))
                    ob = obp.tile([P, 512], F32, tag="ob")
                    nc.vector.tensor_copy(out=ob[:], in_=pd[:])
                    nc.sync.dma_start(
                        out=prm["out_t"][mo * P:(mo + 1) * P, lb * 512:(lb + 1) * 512],
                        in_=ob[:])



def build_nc(repeat=1, phases="abcd"):
    key = (repeat, phases)
    if key in _CACHE:
        return _CACHE[key]
    nc = bacc.Bacc("TRN2", target_bir_lowering=False, debug=False, num_devices=8)
    prm = {}
    for name in ("xq", "xk", "xv"):
        prm[name] = nc.declare_dram_parameter(name, [D, L], F32R, isOutput=False)
    for name in ("wq", "wk", "wv"):
        prm[name] = nc.declare_dram_parameter(name, [D, DO], F32R, isOutput=False)
    prm["wo"] = nc.declare_dram_parameter("wo", [DO, D], F32R, isOutput=False)
    prm["bq"] = nc.declare_dram_parameter("bq", [DO], F32, isOutput=False)
    prm["bk"] = nc.declare_dram_parameter("bk", [DO], F32, isOutput=False)
    prm["indt"] = nc.declare_dram_parameter("indt", [P, 2], F32R, isOutput=False)
    prm["ones8"] = nc.declare_dram_parameter("ones8", [P, 8], F32R, isOutput=False)
    prm["selq"] = nc.declare_dram_parameter("selq", [8, NM, P], F32R, isOutput=False)
    prm["selz"] = nc.declare_dram_parameter("selz", [8, 8, 64], F32R, isOutput=False)
    prm["out_t"] = nc.declare_dram_parameter("out_t", [D, L], F32, isOutput=True)
    _emit(nc, prm, repeat=repeat, phases=phases)
    nc.compile()
    _CACHE[key] = nc
    return nc


def make_in_maps(q, k, v, Wq, bq, Wk, bk, Wv, bv, Wo, bo):
    B = q.shape[0]
    f32 = np.float32

    indt = np.zeros((P, 2), f32)
    indt[0:64, 0] = 1.0
    indt[64:128, 1] = 1.0
    ones8 = np.ones((P, 8), f32)
    selq = np.zeros((8, NM, P), f32)
    for m in range(NM):
        for j in range(P):
            selq[2 * m + j // 64, m, j] = 1.0
    selz = np.zeros((8, 8, 64), f32)
    for r in range(8):
        selz[r, r, :] = 1.0

    in_maps = []
    for b in range(B):
        for g in range(2):
            sl = slice(g * DO, (g + 1) * DO)
            in_maps.append({
                "xq": np.ascontiguousarray(q[b].T.astype(f32)),
                "xk": np.ascontiguousarray(k[b].T.astype(f32)),
                "xv": np.ascontiguousarray(v[b].T.astype(f32)),
                "wq": np.ascontiguousarray(Wq[sl, :].T.astype(f32)),
                "wk": np.ascontiguousarray(Wk[sl, :].T.astype(f32)),
                "wv": np.ascontiguousarray(Wv[sl, :].T.astype(f32)),
                "wo": np.ascontiguousarray(Wo[:, sl].T.astype(f32)),
                "bq": np.ascontiguousarray(bq[sl].astype(f32)),
                "bk": np.ascontiguousarray(bk[sl].astype(f32)),
                "indt": indt, "ones8": ones8, "selq": selq, "selz": selz,
            })
    return in_maps


def assemble(results, bv, Wo, bo):
    B = len(results) // 2
    bias = (bo + bv @ Wo.T).astype(np.float32)
    outs = []
    for b in range(B):
        part = results[2 * b]["out_t"] + results[2 * b + 1]["out_t"]
        outs.append(part.T + bias)
    return np.stack(outs).astype(np.float32)


def kernel(q, k, v, Wq, bq, Wk, bk, Wv, bv, Wo, bo):
    q, k, v = (np.asarray(t, np.float32) for t in (q, k, v))
    Wq, bq, Wk, bk, Wv, bv, Wo, bo = (
        np.asarray(t, np.float32) for t in (Wq, bq, Wk, bk, Wv, bv, Wo, bo))
    nc = build_nc()
    in_maps = make_in_maps(q, k, v, Wq, bq, Wk, bk, Wv, bv, Wo, bo)
    last_err = None
    for attempt in range(3):
        try:
            res = run_bass_kernel_spmd(nc, in_maps, core_ids=list(range(8)))
            return assemble(res.results, bv, Wo, bo)
        except Exception as e:  # transient NRT device errors: retry
            last_err = e
            import time as _time
            _time.sleep(2.0)
    raise last_err


def _emit_norms(nc, tc, normp, psBC, qt, kt, nq_all, nk_all, selq):
    with nc.allow_low_precision(reason="f32r norm chain, tf32 rounding fine"):
        nc.scalar.activation(out=nq_all[:], in_=nq_all[:], func=SQRT)
        nc.vector.tensor_scalar_max(out=nq_all[:], in0=nq_all[:], scalar1=1e-12)
        nc.vector.reciprocal(out=nq_all[:], in_=nq_all[:])
        nc.scalar.activation(out=nk_all[:], in_=nk_all[:], func=SQRT)
        # clamp at eps, then fold the softmax temperature into k's norm
        nc.vector.tensor_scalar_max(out=nk_all[:], in0=nk_all[:], scalar1=1e-12)
        nc.vector.tensor_scalar_mul(out=nk_all[:], in0=nk_all[:], scalar1=TAU)
        nc.vector.reciprocal(out=nk_all[:], in_=nk_all[:])

    # normalize Q and K in place via PE-broadcast of the row pair
    for which, r_all in (("q", nq_all), ("k", nk_all)):
        for m in range(NM):
            for lb in range(NLB):
                sl = slice(lb * 512, (lb + 1) * 512)
                bc = psBC.tile([P, 512], F32, tag="bcq")
                nc.tensor.matmul(bc[:], lhsT=selq[:, m, :], rhs=r_all[:, sl],
                                 start=True, stop=True)
                blk = (qt if which == "q" else kt)[m][:, sl]
                nc.vector.tensor_tensor(out=blk, in0=blk, in1=bc[:], op=MULT)


def _emit_pv(nc, vg, et, lk, ot0, ot1, m):
    nc.tensor.matmul(ot0[:], lhsT=vg[lk][:, 2 * m, :], rhs=et[:, 0:512],
                     start=(lk == 0), stop=(lk == NLK - 1), skip_group_check=True)
    nc.tensor.matmul(ot1[:], lhsT=vg[lk][:, 2 * m + 1, :], rhs=et[:, 512:1024],
                     start=(lk == 0), stop=(lk == NLK - 1), skip_group_check=True)

